# revision 1
# baseline (speedup 1.0000x reference)
"""DSMIL pooling kernel for 8 Trainium2 NeuronCores.

Sharding: B=4 bags x 2-way sequence split of N=16384 -> 8 shards of
[8192, 1024]. Launch 1 computes h^T = we^T @ x^T (+be) per shard on
device (memory-bound patch-embed matmul, f32r). The tiny glue (argmax
candidate selection, q/v projections: ~0.3% of FLOPs) runs on host.
Launch 2 computes attention scores s = h @ (wk@q)/sqrt(E), a local
softmax, and the attention-weighted sum U = sum_n w_n h_n on device.
Host merges the two half-shards per bag (online-softmax combine) and
applies the final [1024,2] head.
"""

import numpy as np

import concourse.mybir as mybir
import concourse.tile as tile
from concourse import bacc
from concourse.bass import ts
from concourse.bass_utils import run_bass_kernel_spmd

# ---- tile-tail drain workaround (this walrus build rejects >1 sync-wait
# per instruction on the kernel-tail Drain) ----
from concourse.vector_clock import ScopedClock

_MAX_WAITS = 1


def _patched_drain_and_barrier(self, tick_clock, wait_clock):
    probe = self.nc.sync.nop(nofuse=True, hint="tile_drain_waits")
    wait_clock.add_sem_waits(probe.ins, ScopedClock({None: tick_clock.global_clock}))
    si = probe.ins.sync_info
    waits = list(si.on_wait) if si is not None and si.on_wait else []
    if len(waits) > _MAX_WAITS:
        si.on_wait = waits[:_MAX_WAITS]
        rest = waits[_MAX_WAITS:]
        for k in range(0, len(rest), _MAX_WAITS):
            extra = self.nc.sync.nop(nofuse=True, hint="tile_drain_waits")
            esi = extra.ins.sync_info
            if esi is None:
                extra.ins.sync_info = mybir.SyncInfo(
                    on_wait=rest[k : k + _MAX_WAITS], on_update=[]
                )
            else:
                esi.on_wait = rest[k : k + _MAX_WAITS]
    self.nc.sync.drain()
    self.nc.all_engine_barrier()
    popped = self.nc._tile_sem_poison_stack.pop()
    assert popped is self._sem_poison
    self.nc.clear_and_free_semaphores(list(self.sems.allocated().values()))
    self.nc.all_engine_barrier()


tile.TileContext._drain_and_barrier = _patched_drain_and_barrier

F32 = mybir.dt.float32
F32R = mybir.dt.float32r

B, N, D, E, C = 4, 16384, 1024, 512, 2
NCORES = 8
NS = N // 2          # per-core sequence shard
NT = 512             # n-tile (psum free dim)
NTILES = NS // NT    # 16
DB = D // 128        # 8
EB = E // 128        # 4

_cache = {}


def _build_launch1():
    nc = bacc.Bacc(None, target_bir_lowering=False)
    xt_d = nc.dram_tensor("xt", [D, NS], F32R, kind="ExternalInput")
    we_d = nc.dram_tensor("we", [D, E], F32R, kind="ExternalInput")
    bec_d = nc.dram_tensor("bec", [128, EB], F32, kind="ExternalInput")
    hT_d = nc.dram_tensor("hT", [128, EB, NS], F32, kind="ExternalOutput")

    with tile.TileContext(nc) as tc:
        with (
            tc.tile_pool(name="wpool", bufs=1) as wp,
            tc.tile_pool(name="xpool", bufs=3) as xp,
            tc.tile_pool(name="hpool", bufs=4) as hp,
            tc.tile_pool(name="psum", bufs=1, space="PSUM") as pp,
        ):
            we_sb = wp.tile([128, DB, E], F32R)
            nc.sync.dma_start(we_sb[:], we_d.rearrange("(db p) e -> p db e", p=128))
            bec_sb = wp.tile([128, EB], F32)
            nc.sync.dma_start(bec_sb[:], bec_d[:])

            xt3 = xt_d.rearrange("(db p) n -> p db n", p=128)

            for ntp in range(NTILES // 2):
                x_t = xp.tile([128, DB, 2 * NT], F32R, tag="xt", name="x_t")
                nc.sync.dma_start(x_t[:], xt3[:, :, ts(ntp, 2 * NT)])
                x_ts = [x_t[:, :, 0:NT], x_t[:, :, NT : 2 * NT]]
                # one weight load serves both n-tiles: loop (eb, db) outer,
                # the two matmuls with identical lhsT run back-to-back
                ps = [
                    [pp.tile([128, NT], F32, tag=f"ph{eb}_{k}", name=f"ph{eb}_{k}") for k in range(2)]
                    for eb in range(EB)
                ]
                for eb in range(EB):
                    for db in range(DB):
                        for k in range(2):
                            nc.tensor.matmul(
                                ps[eb][k][:],
                                lhsT=we_sb[:, db, ts(eb, 128)],
                                rhs=x_ts[k][:, db, :],
                                start=(db == 0),
                                stop=(db == DB - 1),
                            )
                for k in range(2):
                    nt = 2 * ntp + k
                    stg = hp.tile([128, EB, NT], F32, tag="hstg", name="hstg")
                    for eb in range(EB):
                        dst = stg[:, eb, :]
                        if eb % 2 == 0:
                            nc.vector.tensor_scalar(
                                dst, ps[eb][k][:], bec_sb[:, eb : eb + 1], None,
                                op0=mybir.AluOpType.add,
                            )
                        else:
                            nc.scalar.activation(
                                dst, ps[eb][k][:],
                                mybir.ActivationFunctionType.Identity,
                                bias=bec_sb[:, eb : eb + 1], scale=1.0,
                            )
                    nc.sync.dma_start(hT_d[:, :, ts(nt, NT)], stg[:])
    nc.compile()
    return nc


def _build_launch2():
    nc = bacc.Bacc(None, target_bir_lowering=False)
    hT_d = nc.dram_tensor("hT", [128, EB, NS], F32, kind="ExternalInput")
    v_d = nc.dram_tensor("v", [128, EB], F32, kind="ExternalInput")  # v/sqrt(E), blocked
    out_d = nc.dram_tensor("out2", [128, 8], F32, kind="ExternalOutput")

    import concourse.bass_isa as bass_isa

    with tile.TileContext(nc) as tc:
        with (
            tc.tile_pool(name="hpool", bufs=1) as hp,
            tc.tile_pool(name="spool", bufs=1) as sp,
            tc.tile_pool(name="wpool", bufs=3) as wbp,
            tc.tile_pool(name="dram", bufs=1, space="DRAM") as dp,
            tc.tile_pool(name="psum", bufs=4, space="PSUM") as pp,
        ):
            h_tiles = []
            for nt in range(NTILES):
                h_t = hp.tile([128, EB, NT], F32, tag=f"ht{nt}", name=f"ht{nt}")
                nc.sync.dma_start(h_t[:], hT_d[:, :, ts(nt, NT)])
                h_tiles.append(h_t)
            v_sb = sp.tile([128, EB], F32)
            nc.sync.dma_start(v_sb[:], v_d[:])

            s_row = sp.tile([1, NS], F32, tag="rowbuf")
            for nt in range(NTILES):
                ps_s = pp.tile([1, NT], F32, tag="ps")
                for eb in range(EB):
                    nc.tensor.matmul(
                        ps_s[:],
                        lhsT=v_sb[:, eb : eb + 1],
                        rhs=h_tiles[nt][:, eb, :],
                        start=(eb == 0),
                        stop=(eb == EB - 1),
                    )
                nc.vector.tensor_copy(s_row[0:1, ts(nt, NT)], ps_s[:])

            # reshape row -> [128, 64] via DRAM bounce; n = p*64 + j
            row_dram = dp.tile([1, NS], F32)
            nc.sync.dma_start(row_dram[:], s_row[:])
            s2d = sp.tile([128, NS // 128], F32)
            nc.sync.dma_start(
                s2d[:], row_dram.rearrange("a (p j) -> p (a j)", p=128)
            )
            rmax = sp.tile([128, 1], F32)
            nc.vector.reduce_max(rmax[:], s2d[:], axis=mybir.AxisListType.X)
            mx = sp.tile([128, 1], F32)
            nc.gpsimd.partition_all_reduce(
                mx[:], rmax[:], channels=128, reduce_op=bass_isa.ReduceOp.max
            )
            negm = sp.tile([128, 1], F32)
            nc.vector.tensor_scalar_mul(negm[:], mx[:], -1.0)
            w2d = sp.tile([128, NS // 128], F32)
            ssum = sp.tile([128, 1], F32)
            nc.scalar.activation(
                w2d[:], s2d[:], mybir.ActivationFunctionType.Exp,
                bias=negm[:], scale=1.0, accum_out=ssum[:],
            )
            stot = sp.tile([128, 1], F32)
            nc.gpsimd.partition_all_reduce(
                stot[:], ssum[:], channels=128, reduce_op=bass_isa.ReduceOp.add
            )
            # back to a row via DRAM bounce (inverse reshape)
            wrow_dram = dp.tile([1, NS], F32)
            nc.sync.dma_start(
                wrow_dram.rearrange("a (p j) -> p (a j)", p=128), w2d[:]
            )
            w_row = sp.tile([1, NS], F32, tag="rowbuf")
            nc.sync.dma_start(w_row[:], wrow_dram[:])

            u_acc = sp.tile([128, EB], F32)
            nc.vector.memset(u_acc[:], 0.0)
            for nt in range(NTILES):
                w_bc = wbp.tile([128, NT], F32, tag="wbc")
                nc.gpsimd.partition_broadcast(
                    w_bc[:], w_row[0:1, ts(nt, NT)], channels=128
                )
                for eb in range(EB):
                    scr = wbp.tile([128, NT], F32, tag="scr")
                    red = wbp.tile([128, 1], F32, tag="red")
                    nc.vector.tensor_mul(scr[:], h_tiles[nt][:, eb, :], w_bc[:])
                    nc.vector.reduce_sum(red[:], scr[:], axis=mybir.AxisListType.X)
                    nc.vector.tensor_add(
                        u_acc[:, eb : eb + 1], u_acc[:, eb : eb + 1], red[:]
                    )

            out_sb = sp.tile([128, 8], F32)
            nc.vector.memset(out_sb[:], 0.0)
            nc.vector.tensor_copy(out_sb[:, 0:EB], u_acc[:])
            nc.vector.tensor_copy(out_sb[:, 4:5], mx[:])
            nc.vector.tensor_copy(out_sb[:, 5:6], stot[:])
            nc.sync.dma_start(out_d[:], out_sb[:])
    nc.compile()
    return nc


def _blocked(v):
    """[E] -> [128, EB] with out[p, eb] = v[eb*128 + p]."""
    return np.ascontiguousarray(v.reshape(EB, 128).T)


def _unblocked(m):
    """[128, EB] -> [E] inverse of _blocked."""
    return np.ascontiguousarray(m.T.reshape(E))


def kernel(x, we, be, wi, bi, wq, bq, wk, bk, wb, bb):
    x = np.asarray(x, dtype=np.float32)
    we = np.ascontiguousarray(np.asarray(we, dtype=np.float32))
    be = np.asarray(be, dtype=np.float32)
    wi = np.asarray(wi, dtype=np.float32)
    bi = np.asarray(bi, dtype=np.float32)
    wq = np.asarray(wq, dtype=np.float32)
    bq = np.asarray(bq, dtype=np.float32)
    wk = np.asarray(wk, dtype=np.float32)
    bk = np.asarray(bk, dtype=np.float32)
    wb = np.asarray(wb, dtype=np.float32)
    bb = np.asarray(bb, dtype=np.float32)

    if "l1" not in _cache:
        _cache["l1"] = _build_launch1()
    if "l2" not in _cache:
        _cache["l2"] = _build_launch2()

    bec = _blocked(be)

    # per-core shards: core c -> (bag c//2, half c%2)
    in_maps1 = []
    for c in range(NCORES):
        b, h = divmod(c, 2)
        xs = x[b, h * NS : (h + 1) * NS, :]          # [NS, D]
        xt = np.ascontiguousarray(xs.T)              # [D, NS]
        in_maps1.append({"xt": xt, "we": we, "bec": bec})

    res1 = run_bass_kernel_spmd(
        _cache["l1"], in_maps1, core_ids=list(range(NCORES))
    ).results
    hT = [r["hT"] for r in res1]  # each [128, EB, NS]

    # ---- host glue: instance scores -> critical instance -> q, v ----
    # h_c as [NS, E]: h[n, eb*128+p] = hT[p, eb, n]
    h_flat = [
        np.ascontiguousarray(t.transpose(2, 1, 0).reshape(NS, E)) for t in hT
    ]
    scale = np.float32(E) ** 0.5
    v_cols = [None] * NCORES
    crit = [None] * B
    for b in range(B):
        c0, c1 = 2 * b, 2 * b + 1
        best = None
        for c in (c0, c1):
            logits = h_flat[c] @ wi + bi            # [NS, C]
            s = logits.max(axis=1)                   # [NS]
            i = int(s.argmax())
            if best is None or s[i] > best[0]:
                best = (s[i], c, i)
        _, cw, iw = best
        cr = h_flat[cw][iw]                          # [E]
        crit[b] = cr
        q = cr @ wq + bq                             # [E]
        v = (wk @ q) / scale                         # [E]
        vc = _blocked(v.astype(np.float32))
        v_cols[c0] = vc
        v_cols[c1] = vc

    in_maps2 = [
        {"hT": np.ascontiguousarray(hT[c]), "v": v_cols[c]} for c in range(NCORES)
    ]
    res2 = run_bass_kernel_spmd(
        _cache["l2"], in_maps2, core_ids=list(range(NCORES))
    ).results

    # ---- host combine: online softmax across the two halves of each bag ----
    out = np.zeros((B, C), dtype=np.float32)
    for b in range(B):
        parts = []
        for c in (2 * b, 2 * b + 1):
            o = res2[c]["out2"]
            U = _unblocked(o[:, 0:EB])               # [E]
            m = float(o[0, 4])
            S = float(o[0, 5])
            parts.append((m, S, U))
        m_star = max(p[0] for p in parts)
        S_tot = 0.0
        U_tot = np.zeros(E, dtype=np.float64)
        for m, S, U in parts:
            f = np.exp(m - m_star)
            S_tot += S * f
            U_tot += U.astype(np.float64) * f
        attn_bag = (U_tot / S_tot).astype(np.float32)
        fused = np.concatenate([crit[b], attn_bag])  # [2E]
        out[b] = fused @ wb + bb
    return out



# revision 3
# speedup vs baseline: 2.3724x; 2.3724x over previous
"""DSMIL pooling kernel for 8 Trainium2 NeuronCores — folded-algebra fp8 design.

Algebraic folding (exact): with h = x@we + be,
  inst_logits = h@wi + bi           = x @ (we@wi) + (be@wi + bi)
  scores      = q·(h@wk + bk)/√E    = x @ (we@(wk·q))/√E + const  (const cancels in softmax)
  attn_bag    = Σ attn_n h_n        = (Σ attn_n x_n) @ we + be    (Σ attn_n = 1)
So the device only needs three thin, memory-bound x-contractions:
  L1: l = x @ wei      (per-instance logits; host does argmax + exact f32
      recheck of the top-32 noisy candidates, then critical/q/v folds)
  L2: s = x @ wev  ->  w = exp(s)  ->  wX = Σ w_n x_n   (one launch)
x is streamed in fp8e4m3 (x·16), weights fp8 (wei·512, wev·1024); the host
applies exact power-of-2 unscaling. DoubleRow matmuls contract 256 rows per
instruction (dual-fp8 ldweights need stationary free ≥16 and small dual-dim
strides, hence the padded wei and the n = blk*256 + i*128 + p blocking).
Host glue is O(E²) weight folds plus one O(32·D) recheck.

Sharding: core c <- (bag c//2, half c%2), each shard NS=8192 instances.
"""

import numpy as np
import ml_dtypes

import concourse.mybir as mybir
import concourse.tile as tile
from concourse import bacc
from concourse.bass_utils import run_bass_kernel_spmd

# ---- tile-tail drain workaround (this walrus build rejects >1 sync-wait
# per instruction on the kernel-tail Drain) ----
from concourse.vector_clock import ScopedClock

_MAX_WAITS = 1


def _patched_drain_and_barrier(self, tick_clock, wait_clock):
    probe = self.nc.sync.nop(nofuse=True, hint="tile_drain_waits")
    wait_clock.add_sem_waits(probe.ins, ScopedClock({None: tick_clock.global_clock}))
    si = probe.ins.sync_info
    waits = list(si.on_wait) if si is not None and si.on_wait else []
    if len(waits) > _MAX_WAITS:
        si.on_wait = waits[:_MAX_WAITS]
        rest = waits[_MAX_WAITS:]
        for k in range(0, len(rest), _MAX_WAITS):
            extra = self.nc.sync.nop(nofuse=True, hint="tile_drain_waits")
            esi = extra.ins.sync_info
            if esi is None:
                extra.ins.sync_info = mybir.SyncInfo(
                    on_wait=rest[k : k + _MAX_WAITS], on_update=[]
                )
            else:
                esi.on_wait = rest[k : k + _MAX_WAITS]
    self.nc.sync.drain()
    self.nc.all_engine_barrier()
    popped = self.nc._tile_sem_poison_stack.pop()
    assert popped is self._sem_poison
    self.nc.clear_and_free_semaphores(list(self.sems.allocated().values()))
    self.nc.all_engine_barrier()


tile.TileContext._drain_and_barrier = _patched_drain_and_barrier

F32 = mybir.dt.float32
F8 = mybir.dt.float8e4
FP8 = ml_dtypes.float8_e4m3

B, N, D, E, C = 4, 16384, 1024, 512, 2
NCORES = 8
NS = N // 2          # per-core sequence shard
NT = 512             # n-tile for the xT stream
NTILES = NS // NT    # 16
PR = 4               # 256-row DoubleRow blocks along D (d = pr*256 + i*128 + p)
NBLK = NS // 256     # 32  (n = blk*256 + i*128 + p)
DC = D // 128        # 8
CPAD = 16            # wei columns padded up to the dual-fp8 ldweights minimum

XS = 16.0            # x fp8 scale
WEIS = 512.0         # wei fp8 scale
WEVS = 1024.0        # wev fp8 scale

_cache = {}


def _build_l1():
    nc = bacc.Bacc(None, target_bir_lowering=False)
    xt_d = nc.dram_tensor("xt", [128, NTILES, PR, 2, NT], F8, kind="ExternalInput")
    wei_d = nc.dram_tensor("wei", [128, PR, 2, CPAD], F8, kind="ExternalInput")
    l_d = nc.dram_tensor("l", [C, NTILES, NT], F32, kind="ExternalOutput")

    with tile.TileContext(nc) as tc:
        with (
            tc.tile_pool(name="wp", bufs=1) as wp,
            tc.tile_pool(name="xp", bufs=3) as xp,
            tc.tile_pool(name="ps", bufs=4, space="PSUM") as pp,
        ):
            wei_sb = wp.tile([128, PR, 2, CPAD], F8)
            nc.sync.dma_start(wei_sb[:], wei_d[:])
            lstack = wp.tile([C, NTILES, NT], F32)

            for k in range(NTILES // 2):
                x_t = xp.tile([128, 2, PR, 2, NT], F8, tag="xt", name="xt")
                nc.sync.dma_start(x_t[:], xt_d[:, 2 * k : 2 * k + 2])
                for t2 in range(2):
                    nt = 2 * k + t2
                    ps = pp.tile([CPAD, NT], F32, tag="l")
                    for pr in range(PR):
                        nc.tensor.matmul(
                            ps[:],
                            lhsT=wei_sb[:, pr],
                            rhs=x_t[:, t2, pr],
                            start=(pr == 0),
                            stop=(pr == PR - 1),
                            perf_mode=mybir.MatmulPerfMode.DoubleRow,
                        )
                    nc.scalar.copy(lstack[:, nt, :], ps[0:C, :])
            nc.sync.dma_start(l_d[:], lstack[:])
    nc.compile()
    return nc


def _build_l2():
    nc = bacc.Bacc(None, target_bir_lowering=False)
    xt_d = nc.dram_tensor("xt", [128, NTILES, PR, 2, NT], F8, kind="ExternalInput")
    xn_d = nc.dram_tensor("xn", [128, NBLK, 2, D], F8, kind="ExternalInput")
    wev_d = nc.dram_tensor("wev", [128, PR, 2, 1], F8, kind="ExternalInput")
    u_d = nc.dram_tensor("u", [128, DC], F32, kind="ExternalOutput")
    ssum_d = nc.dram_tensor("ssum", [128, 1], F32, kind="ExternalOutput")

    with tile.TileContext(nc) as tc:
        with (
            tc.tile_pool(name="wp", bufs=1) as wp,
            tc.tile_pool(name="xp", bufs=3) as xp,
            tc.tile_pool(name="pw", bufs=1, space="PSUM") as pwp,
        ):
            wev_sb = wp.tile([128, PR, 2, 1], F8)
            nc.sync.dma_start(wev_sb[:], wev_d[:])

            # ---- phase A: s = x @ wev, directly in n-partition layout.
            # lhsT = x-chunk (stationary), rhs = wev (moving):
            # wps[p, i, blk] = s[blk*256 + i*128 + p] (raw, scaled by XS*WEVS).
            wps = pwp.tile([128, 2, NBLK, 1], F32, tag="w")
            for k in range(NTILES // 2):
                x_t = xp.tile([128, 2, PR, 2, NT], F8, tag="xt", name="xt")
                nc.sync.dma_start(x_t[:], xt_d[:, 2 * k : 2 * k + 2])
                for t2 in range(2):
                    nt = 2 * k + t2
                    for jc in range(4):
                        for pr in range(PR):
                            nc.tensor.matmul(
                                wps[:, jc % 2, 2 * nt + jc // 2],
                                lhsT=x_t[:, t2, pr, :, jc * 128 : (jc + 1) * 128],
                                rhs=wev_sb[:, pr],
                                start=(pr == 0),
                                stop=(pr == PR - 1),
                                perf_mode=mybir.MatmulPerfMode.DoubleRow,
                            )

            # xn streamed in chunks behind the xt tiles (same DMA ring)
            xn_sb = wp.tile([128, NBLK, 2, D], F8)
            for ch in range(4):
                nc.sync.dma_start(
                    xn_sb[:, ch * (NBLK // 4) : (ch + 1) * (NBLK // 4)],
                    xn_d[:, ch * (NBLK // 4) : (ch + 1) * (NBLK // 4)],
                )

            # ---- w = exp(s/16384) quantized to fp8, with per-partition sums ----
            w2d = wp.tile([128, 2, NBLK, 1], F8)
            ssum = wp.tile([128, 1], F32)
            nc.scalar.activation(
                w2d[:], wps[:], mybir.ActivationFunctionType.Exp,
                scale=1.0 / (XS * WEVS), accum_out=ssum[:],
            )
            nc.sync.dma_start(ssum_d[:], ssum[:])

            # ---- phase B: wX = Σ w_n x_n, u[p, dc] = wX[dc*128 + p] ----
            # blk outer so matmuls chase the xn chunk DMAs; the 8 psum
            # accumulation groups (one per dc) interleave, which is fine on
            # hardware (accumulate targets its own bank address).
            pu = pwp.tile([128, DC, 1], F32, tag="u")
            for blk in range(NBLK):
                for dc in range(DC):
                    nc.tensor.matmul(
                        pu[:, dc],
                        lhsT=xn_sb[:, blk, :, dc * 128 : (dc + 1) * 128],
                        rhs=w2d[:, :, blk],
                        start=(blk == 0),
                        stop=(blk == NBLK - 1),
                        perf_mode=mybir.MatmulPerfMode.DoubleRow,
                        skip_group_check=True,
                    )
            u_sb = wp.tile([128, DC], F32)
            nc.scalar.copy(u_sb[:], pu[:, :, 0])
            nc.sync.dma_start(u_d[:], u_sb[:])
    nc.compile()
    return nc


def _q8(a, scale, lim=200.0):
    return np.clip(np.asarray(a, np.float32) * scale, -lim, lim).astype(FP8)


def _prep_x(xs):
    """xs [NS, D] f32 -> (xt8 [128, NTILES, PR, 2, NT], xn8 [128, NBLK, 2, D])."""
    xq = _q8(xs, XS)
    xt8 = np.ascontiguousarray(
        xq.reshape(NTILES, NT, PR, 2, 128).transpose(4, 0, 2, 3, 1)
    )
    xn8 = np.ascontiguousarray(
        xq.reshape(NBLK, 2, 128, D).transpose(2, 0, 1, 3)
    )
    return xt8, xn8


def _blk_d(v, scale):
    """[D, m] f32 -> [128, PR, 2, m] fp8 with d = pr*256 + i*128 + p."""
    v = np.asarray(v, np.float32)
    if v.ndim == 1:
        v = v[:, None]
    m = v.shape[1]
    return np.ascontiguousarray(
        _q8(v, scale).reshape(PR, 2, 128, m).transpose(2, 0, 1, 3)
    )


def kernel(x, we, be, wi, bi, wq, bq, wk, bk, wb, bb):
    x = np.asarray(x, dtype=np.float32)
    we = np.asarray(we, dtype=np.float32)
    be = np.asarray(be, dtype=np.float32)
    wi = np.asarray(wi, dtype=np.float32)
    bi = np.asarray(bi, dtype=np.float32)
    wq = np.asarray(wq, dtype=np.float32)
    bq = np.asarray(bq, dtype=np.float32)
    wk = np.asarray(wk, dtype=np.float32)
    bk = np.asarray(bk, dtype=np.float32)
    wb = np.asarray(wb, dtype=np.float32)
    bb = np.asarray(bb, dtype=np.float32)

    if "l1" not in _cache:
        _cache["l1"] = _build_l1()
    if "l2" not in _cache:
        _cache["l2"] = _build_l2()

    wei = we @ wi                       # [D, C]
    bei = be @ wi + bi                  # [C]
    wei_pad = np.zeros((D, CPAD), np.float32)
    wei_pad[:, :C] = wei
    wei8 = _blk_d(wei_pad, WEIS)

    shards = []                         # per-core (xt8, xn8)
    for c in range(NCORES):
        b, h = divmod(c, 2)
        shards.append(_prep_x(x[b, h * NS : (h + 1) * NS]))

    in1 = [{"xt": s[0], "wei": wei8} for s in shards]
    res1 = run_bass_kernel_spmd(_cache["l1"], in1, core_ids=list(range(NCORES))).results

    # ---- host glue: noisy argmax + exact f32 recheck -> critical -> v ----
    scale = np.float32(E) ** 0.5
    crit = [None] * B
    wev8s = [None] * NCORES
    for b in range(B):
        sc_parts = []
        for h in range(2):
            lraw = res1[2 * b + h]["l"]              # [C, NTILES, NT]
            l = lraw.transpose(1, 2, 0).reshape(NS, C) / (XS * WEIS) + bei
            sc_parts.append(l.max(axis=1))
        sc = np.concatenate(sc_parts)                # [N] noisy instance scores
        cand = np.argpartition(sc, -64)[-64:]
        lex = x[b][cand] @ wei + bei                 # exact f32 recheck
        i = int(cand[int(lex.max(axis=1).argmax())])
        cr = x[b, i] @ we + be                       # exact critical embedding
        crit[b] = cr
        q = cr @ wq + bq
        v = (wk @ q) / scale
        wev = we @ v                                 # [D]
        w8 = _blk_d(wev, WEVS)
        wev8s[2 * b] = w8
        wev8s[2 * b + 1] = w8

    in2 = [
        {"xt": shards[c][0], "xn": shards[c][1], "wev": wev8s[c]}
        for c in range(NCORES)
    ]
    res2 = run_bass_kernel_spmd(_cache["l2"], in2, core_ids=list(range(NCORES))).results

    out = np.zeros((B, C), dtype=np.float32)
    for b in range(B):
        # u[p, dc] = Σ_n w_n x8[n, dc*128+p]; halves of the bag add
        u = (
            res2[2 * b]["u"].astype(np.float64) + res2[2 * b + 1]["u"]
        ).T.reshape(D)
        S = float(
            res2[2 * b]["ssum"].sum(dtype=np.float64)
            + res2[2 * b + 1]["ssum"].sum(dtype=np.float64)
        )
        wX = u / (XS * S)
        attn_bag = wX @ we + be
        fused = np.concatenate([crit[b], attn_bag])
        out[b] = fused @ wb + bb
    return out
